# revision 17
# baseline (speedup 1.0000x reference)
"""Trainium2 kernel v4 for nn_GravityHypothesisTester — fold-max residue argmax.

Per core (batch b = c%4, orientation o = c//4) the N x N score matrix
G[n,m] = 2*s_n.t_m - (yy_m - mean(yy)) is built in PSUM by fp16 hi/lo split
matmuls (K=14, exact to ~2^-22 relative). Per [128,4096] row-tile:

  - ACT casts each PSUM half [128,2048] fp32 -> fp16 M in SBUF (the drain)
  - DVE fold-max cascade per half: 2048 -> 1024 -> 512 -> 256 via
    tensor_tensor(max) on contiguous halves (2x_1p fast mode). M256[256h+j]
    = max over columns {2048h + j + 256k, k=0..7} (a residue class).
  - one 4x tensor_scalar over the concatenated [128,512] fold gives the
    row max v (exact fp16 value, fp32 accum)
  - one scalar_tensor_tensor (M256 is_ge v) * W with sum-accum gives
    S = W[c] = 512-c at the first residue achieving v

Host decodes c = 512-S -> (half, j), recomputes exact fp32 G on the 8
candidate columns of that residue, and picks the true argmax. Rows where S
is invalid (fp16 ties across residues) or the refined max mismatches v are
recomputed exactly on host (~1% of rows). Host does the tiny O(B*N)
pre/post work (Rodrigues, means, median, sigmoid).
"""

import sys
from contextlib import ExitStack

import numpy as np

sys.path.insert(0, "/opt/trn_rl_repo")

import concourse.bass as bass
import concourse.tile as tile
from concourse import bacc, mybir
from concourse.bass_utils import run_bass_kernel_spmd

EPS = 1e-6
CHI2_THRESH = 9.0
DIST_SCALE = 3.0
B = 4
N = 4096
P = 128
ROW_TILES = N // P          # 32
HALF = 2048
MMCOLS = 512
N_CORES = 8
KDIM = 14                   # fp16 split rows: 4x3 products + 2 ones rows
RES = 256                   # residues mod 256 over the full 4096-wide row
NFOLD = N // RES            # 16 candidate columns per residue class
DVE_DRAIN_MOD = 9           # every 9th half drains via DVE instead of ACT

_NC_CACHE = {}


def _build_nc(repeat=1, ablate=0):
    assert ablate == 0
    key = ("nc", repeat)
    if key in _NC_CACHE:
        return _NC_CACHE[key]

    nc = bacc.Bacc("TRN2", target_bir_lowering=False)
    f32 = mybir.dt.float32
    f16 = mybir.dt.float16

    lhsT_d = nc.dram_tensor("lhsT", [KDIM, N], f16, kind="ExternalInput")
    rhs_d = nc.dram_tensor("rhs", [KDIM, N], f16, kind="ExternalInput")
    wramp_d = nc.dram_tensor("wramp", [P, RES], f16, kind="ExternalInput")
    vstage_d = nc.dram_tensor("vstage", [P, ROW_TILES], f32,
                              kind="ExternalOutput")
    sstage_d = nc.dram_tensor("sstage", [P, ROW_TILES], f32,
                              kind="ExternalOutput")

    with tile.TileContext(nc) as tc, ExitStack() as ctx:
        inp = ctx.enter_context(tc.tile_pool(name="inp", bufs=1))
        stage = ctx.enter_context(tc.tile_pool(name="stage", bufs=1))
        mpool = ctx.enter_context(tc.tile_pool(name="mpool", bufs=2))
        fpool = ctx.enter_context(tc.tile_pool(name="fpool", bufs=2))
        psum = ctx.enter_context(tc.tile_pool(name="psum", bufs=2, space="PSUM"))

        lhsT_s = inp.tile([KDIM, N], f16)
        rhs_s = inp.tile([KDIM, N], f16)
        wramp_s = inp.tile([P, RES], f16)
        # Split input DMAs so the first row-tile's matmuls wait only on the
        # small head pieces, not the full tensors.
        nc.sync.dma_start(lhsT_s[:, 0:P], lhsT_d[:, 0:P])
        nc.sync.dma_start(rhs_s[:, 0:HALF], rhs_d[:, 0:HALF])
        nc.sync.dma_start(lhsT_s[:, P:N], lhsT_d[:, P:N])
        nc.sync.dma_start(rhs_s[:, HALF:N], rhs_d[:, HALF:N])
        nc.sync.dma_start(wramp_s[:], wramp_d[:])

        vstage = stage.tile([P, ROW_TILES], f32)
        sstage = stage.tile([P, ROW_TILES], f32)
        junk_max = stage.tile([P, RES], f16)
        junk_stt = stage.tile([P, RES], f16)

        half_idx = 0
        for t in range(ROW_TILES * repeat):
            rt = t % ROW_TILES
            m16 = mpool.tile([P, N], f16)       # fp16 matrix row-tile
            m2 = fpool.tile([P, N // 2], f16)   # fold 1: mod-1024 per half
            m4 = fpool.tile([P, N // 4], f16)   # fold 2: mod-1024 both halves
            m8 = fpool.tile([P, N // 8], f16)   # fold 3: mod-512
            mr = fpool.tile([P, RES], f16)      # fold 4: mod-256 full row
            for h in range(2):
                pt = psum.tile([P, HALF], f32)
                if t == 0 and h == 0:
                    # Dummy matmul reading only rhs_s: the PE weight-load
                    # HW slot carries a single semaphore wait, so the two
                    # input DMA waits must land on separate PE instructions.
                    nc.tensor.matmul(
                        pt[:, 0:MMCOLS], rhs_s[:, 0:P], rhs_s[:, 0:MMCOLS],
                        start=True, stop=True,
                    )
                for j in range(HALF // MMCOLS):
                    nc.tensor.matmul(
                        pt[:, bass.ts(j, MMCOLS)],
                        lhsT_s[:, bass.ts(rt, P)],
                        rhs_s[:, h * HALF + j * MMCOLS
                              : h * HALF + (j + 1) * MMCOLS],
                        start=True,
                        stop=True,
                    )
                # drain: cast fp32 PSUM -> fp16 SBUF; mostly on ACT, every
                # DVE_DRAIN_MOD-th half on DVE to balance the engines
                off = h * HALF
                if half_idx % DVE_DRAIN_MOD == DVE_DRAIN_MOD - 1:
                    nc.vector.tensor_scalar(
                        m16[:, off : off + HALF], pt[:], 0.0, None,
                        op0=mybir.AluOpType.add)
                else:
                    nc.scalar.copy(m16[:, off : off + HALF], pt[:])
                # first fold on DVE (2x_1p fp16)
                nc.vector.tensor_tensor(
                    m2[:, h * 1024 : h * 1024 + 1024],
                    m16[:, off : off + 1024],
                    m16[:, off + 1024 : off + 2048],
                    op=mybir.AluOpType.max,
                )
                half_idx += 1
            # remaining folds merge the two halves: residues mod 256 over
            # the full 4096-wide row
            nc.vector.tensor_tensor(
                m4[:], m2[:, 0:1024], m2[:, 1024:2048], op=mybir.AluOpType.max)
            nc.vector.tensor_tensor(
                m8[:], m4[:, 0:512], m4[:, 512:1024], op=mybir.AluOpType.max)
            nc.vector.tensor_tensor(
                mr[:], m8[:, 0:256], m8[:, 256:512], op=mybir.AluOpType.max)
            # row max v (4x fp16; fp32 accum holds the exact fp16 value)
            nc.vector.tensor_scalar(
                junk_max[:], mr[:], 0.0, None,
                op0=mybir.AluOpType.add, op1=mybir.AluOpType.max,
                accum_out=vstage[:, rt : rt + 1],
            )
            # S = sum of W over residues achieving v (single hit: S = 256-j)
            nc.vector.scalar_tensor_tensor(
                junk_stt[:], mr[:], vstage[:, rt : rt + 1], wramp_s[:],
                op0=mybir.AluOpType.is_ge, op1=mybir.AluOpType.mult,
                accum_out=sstage[:, rt : rt + 1],
            )

        nc.sync.dma_start(vstage_d[:], vstage[:])
        nc.sync.dma_start(sstage_d[:], sstage[:])

    nc.finalize()
    _NC_CACHE[key] = nc
    return nc


def _get_runner(repeat=1, ablate=0):
    """Build the sharded PJRT executable once; reuse across kernel() calls."""
    rkey = ("runner", repeat, ablate)
    if rkey in _NC_CACHE:
        return _NC_CACHE[rkey]

    import jax
    from jax.sharding import Mesh, PartitionSpec
    from jax.experimental.shard_map import shard_map
    from concourse import bass2jax

    nc = _build_nc(repeat, ablate)
    bass2jax.install_neuronx_cc_hook()

    partition_name = nc.partition_id_tensor.name if nc.partition_id_tensor else None
    in_names, out_names, out_avals, zero_outs = [], [], [], []
    for alloc in nc.m.functions[0].allocations:
        if not isinstance(alloc, mybir.MemoryLocationSet):
            continue
        name = alloc.memorylocations[0].name
        if alloc.kind == "ExternalInput":
            if name != partition_name:
                in_names.append(name)
        elif alloc.kind == "ExternalOutput":
            shape = tuple(alloc.tensor_shape)
            np_dt = mybir.dt.np(alloc.dtype)
            out_names.append(name)
            out_avals.append(jax.core.ShapedArray(shape, np_dt))
            zero_outs.append(np.zeros(shape, np_dt))

    n_params = len(in_names)
    n_outs = len(out_names)
    all_in_names = list(in_names) + list(out_names)
    if partition_name is not None:
        all_in_names.append(partition_name)
    donate = tuple(range(n_params, n_params + n_outs))

    def _body(*args):
        operands = list(args)
        if partition_name is not None:
            operands.append(bass2jax.partition_id_tensor())
        outs = bass2jax._bass_exec_p.bind(
            *operands,
            out_avals=tuple(out_avals),
            in_names=tuple(all_in_names),
            out_names=tuple(out_names),
            lowering_input_output_aliases=(),
            sim_require_finite=True,
            sim_require_nnan=True,
            nc=nc,
        )
        return tuple(outs)

    devices = jax.devices()[:N_CORES]
    mesh = Mesh(np.asarray(devices), ("core",))
    in_specs = (PartitionSpec("core"),) * (n_params + n_outs)
    out_specs = (PartitionSpec("core"),) * n_outs
    sharded = jax.jit(
        shard_map(_body, mesh=mesh, in_specs=in_specs, out_specs=out_specs,
                  check_rep=False),
        donate_argnums=donate, keep_unused=True,
    )

    def run(in_maps):
        concat_in = [
            np.concatenate([np.asarray(m[name]) for m in in_maps], axis=0)
            for name in in_names
        ]
        concat_zeros = [
            np.zeros((N_CORES * z.shape[0], *z.shape[1:]), z.dtype)
            for z in zero_outs
        ]
        out_arrs = sharded(*concat_in, *concat_zeros)
        return [
            {
                name: np.asarray(out_arrs[k]).reshape(
                    N_CORES, *out_avals[k].shape)[c]
                for k, name in enumerate(out_names)
            }
            for c in range(N_CORES)
        ]

    _NC_CACHE[rkey] = run
    return run


# ---------------- host-side numpy port of the tiny reference pieces ----------


def _normalize(x, axis, eps=EPS):
    n = np.linalg.norm(x, axis=axis, keepdims=True)
    return x / np.maximum(n, eps)


def _skew(k):
    kx, ky, kz = k[:, 0], k[:, 1], k[:, 2]
    O = np.zeros_like(kx)
    row0 = np.stack([O, -kz, ky], axis=1)
    row1 = np.stack([kz, O, -kx], axis=1)
    row2 = np.stack([-ky, kx, O], axis=1)
    return np.stack([row0, row1, row2], axis=1)


def _gravity_align(g_src, g_tgt, eps=EPS):
    u = _normalize(g_src, 1, eps)
    v = _normalize(g_tgt, 1, eps)
    axis = np.cross(u, v)
    axis_norm = np.linalg.norm(axis, axis=1, keepdims=True)
    dot = np.clip(np.sum(u * v, axis=1, keepdims=True), -1.0, 1.0)
    parallel = axis_norm < 1e-6
    k = axis / (axis_norm + eps)
    theta = np.arccos(dot)
    sin_t, cos_t = np.sin(theta), np.cos(theta)
    K = _skew(k)
    I = np.eye(3, dtype=g_src.dtype)[None]
    R = I + sin_t[:, :, None] * K + (1.0 - cos_t)[:, :, None] * (K @ K)
    ex = np.array([1.0, 0.0, 0.0], dtype=u.dtype)[None]
    ey = np.array([0.0, 1.0, 0.0], dtype=u.dtype)[None]
    use_ex = np.abs(u[:, 0:1]) < 0.9
    basis = np.where(use_ex, ex, ey)
    axis2 = _normalize(np.cross(u, basis), 1, eps)
    K2 = _skew(axis2)
    R_anti = I + 2.0 * (K2 @ K2)
    antipar = parallel & (dot < 0.0)
    R = np.where(antipar[:, :, None], R_anti, R)
    R = np.where((parallel & (dot > 0.0))[:, :, None], I, R)
    return R.astype(np.float32)


def _split16(x):
    hi = x.astype(np.float16)
    lo = (x - hi.astype(np.float32)).astype(np.float16)
    return hi, lo


def _make_operands(v, t, z):
    """lhsT/rhs fp16 rows for G = v.t + z_m (v = 2*s; z per-column)."""
    vh, vl = _split16(v)          # (3, N) each
    th, tl = _split16(t)
    zh, zl = _split16(z)          # (N,)
    ones = np.ones((1, N), np.float16)
    lhsT = np.concatenate([vh, vh, vl, vl, ones, ones], axis=0)
    rhs = np.concatenate([th, tl, th, tl, zh[None], zl[None]], axis=0)
    return (np.ascontiguousarray(lhsT, np.float16),
            np.ascontiguousarray(rhs, np.float16))


_WRAMP = np.ascontiguousarray(
    np.broadcast_to((RES - np.arange(RES, dtype=np.float32))
                    .astype(np.float16)[None, :], (P, RES)))


def _prepare(src, tgt, src_n, tgt_n, g_p, k_p, g_q, k_q):
    src = np.asarray(src, np.float32)
    tgt = np.asarray(tgt, np.float32)
    g_p = np.asarray(g_p, np.float32)
    g_q = np.asarray(g_q, np.float32)

    R_g = _gravity_align(g_p, g_q)
    src_rot = np.einsum("bij,bjn->bin", R_g, src).astype(np.float32)
    t_center = tgt.mean(axis=2, keepdims=True) - src_rot.mean(axis=2, keepdims=True)
    s = (src_rot + t_center).astype(np.float32)  # src_init

    xx = np.sum(s * s, axis=1)  # [B, N]
    yy = np.sum(tgt * tgt, axis=1)
    ybar = yy.mean(axis=1)      # [B]
    xbar = xx.mean(axis=1)

    in_maps = []
    for c in range(N_CORES):
        b, o = c % B, c // B
        if o == 0:
            lhsT, rhs = _make_operands(2.0 * s[b], tgt[b], -(yy[b] - ybar[b]))
        else:
            lhsT, rhs = _make_operands(2.0 * tgt[b], s[b], -(xx[b] - xbar[b]))
        in_maps.append({"lhsT": lhsT, "rhs": rhs, "wramp": _WRAMP})
    return in_maps, s, xx, yy, ybar, xbar


def _prepare_in_maps(src, tgt, src_n, tgt_n, g_p, k_p, g_q, k_q):
    return _prepare(src, tgt, src_n, tgt_n, g_p, k_p, g_q, k_q)[0]


def _decode_core(vstage, sstage, sv, tv, z):
    """Decode one core's [P, 32] staging -> (gmax[N], argidx[N]).

    Row n = 128*rt + p. S = 256-j at the winning residue j (mod 256 over
    the full row); candidates are columns j + 256k, k=0..15. Exact fp32 G
    on the candidates picks the true argmax; rows with invalid S (fp16 ties
    across residues) or a refined max that does not reproduce v are
    recomputed exactly.

    sv: (3, N) row vectors (2*s scaling folded in), tv: (3, N), z: (N,).
    """
    v = vstage.T.reshape(N)             # n = 128*rt + p -> [rt, p] -> flat
    S = sstage.T.reshape(N)
    c = RES - S
    ci = np.rint(c).astype(np.int64)
    valid = (S >= 1) & (S <= RES) & (c == ci)
    ci = np.clip(ci, 0, RES - 1)
    # candidate columns [N, NFOLD]
    cand = ci[:, None] + RES * np.arange(NFOLD)[None, :]
    # exact scores at candidates: G[n, m] = sv[:,n].tv[:,m] + z[m]
    tc = tv[:, cand]                    # (3, N, NFOLD)
    Gc = np.einsum("cn,cnk->nk", sv, tc) + z[cand]
    kbest = np.argmax(Gc, axis=1)
    rows = np.arange(N)
    gbest = Gc[rows, kbest]
    idx = cand[rows, kbest]
    bad = (~valid) | (gbest.astype(np.float16).astype(np.float32) != v)
    if np.any(bad):
        for n in np.where(bad)[0]:
            row = sv[:, n] @ tv + z
            idx[n] = int(np.argmax(row))
            gbest[n] = row[idx[n]]
    return gbest.astype(np.float32), idx


def _sigmoid(x):
    out = np.empty_like(x)
    pos = x >= 0
    out[pos] = 1.0 / (1.0 + np.exp(-x[pos]))
    ex = np.exp(x[~pos])
    out[~pos] = ex / (1.0 + ex)
    return out


def kernel(src, tgt, src_n, tgt_n, g_p, k_p, g_q, k_q):
    src = np.asarray(src, np.float32)
    tgt = np.asarray(tgt, np.float32)
    src_n = np.asarray(src_n, np.float32)
    tgt_n = np.asarray(tgt_n, np.float32)
    g_p = np.asarray(g_p, np.float32)
    g_q = np.asarray(g_q, np.float32)
    k_p = np.asarray(k_p, np.float32)
    k_q = np.asarray(k_q, np.float32)

    in_maps, s, xx, yy, ybar, xbar = _prepare(
        src, tgt, src_n, tgt_n, g_p, k_p, g_q, k_q)
    R_g = _gravity_align(g_p, g_q)
    src_n_rot = np.einsum("bij,bjn->bin", R_g, src_n).astype(np.float32)

    results = _get_runner()(in_maps)

    min_pq = np.empty((B, N), np.float32)
    corr_p2q = np.empty((B, N), np.int64)
    min_qp = np.empty((B, N), np.float32)
    corr_q2p = np.empty((B, N), np.int64)
    for c in range(N_CORES):
        b, o = c % B, c // B
        if o == 0:
            sv, tv, z = 2.0 * s[b], tgt[b], -(yy[b] - ybar[b])
        else:
            sv, tv, z = 2.0 * tgt[b], s[b], -(xx[b] - xbar[b])
        gmax, idx = _decode_core(results[c]["vstage"], results[c]["sstage"],
                                 sv, tv, z)
        if o == 0:
            min_pq[b] = xx[b] - gmax + ybar[b]
            corr_p2q[b] = idx
        else:
            min_qp[b] = yy[b] - gmax + xbar[b]
            corr_q2p[b] = idx

    nn_d_p = np.sqrt(np.maximum(min_pq, 0.0) + EPS)
    nn_d_q = np.sqrt(np.maximum(min_qp, 0.0) + EPS)
    tau_p = DIST_SCALE * np.sort(nn_d_p, axis=1)[:, (N - 1) // 2][:, None]
    tau_q = DIST_SCALE * np.sort(nn_d_q, axis=1)[:, (N - 1) // 2][:, None]
    geom_p = (nn_d_p <= tau_p).astype(np.float32)
    geom_q = (nn_d_q <= tau_q).astype(np.float32)

    gq = g_q[:, :, None]
    inc_p = np.sum(src_n_rot * gq, axis=1)  # [B, N]
    inc_q = np.sum(tgt_n * gq, axis=1)
    inc_p_ref = np.take_along_axis(inc_q, corr_p2q, axis=1)
    inc_q_ref = np.take_along_axis(inc_p, corr_q2p, axis=1)

    k_eff = k_p * k_q / (k_p + k_q + EPS)  # [B,1]
    w_p = _sigmoid(CHI2_THRESH - k_eff * (inc_p - inc_p_ref) ** 2) * geom_p
    w_q = _sigmoid(CHI2_THRESH - k_eff * (inc_q - inc_q_ref) ** 2) * geom_q
    return w_p[:, None, :].astype(np.float32), w_q[:, None, :].astype(np.float32)
